# revision 1
# baseline (speedup 1.0000x reference)
"""3-layer GraphSAGE (mean aggr + L2 norm) on 8 Trainium2 NeuronCores.

Strategy (graph/data parallel, dst-sharded):
  - Nodes are partitioned into 8 contiguous ranges (12500 per core); each core
    computes all three layers for its destination range.
  - Mean aggregation h_agg = D^-1 A h is computed per core as a stream of
    one-hot scatter matmuls: messages h[src] are gathered from a replicated
    fp16 node table in HBM with dma_gather (int16 indices -> 4 source chunks
    of 25000 rows), then each 128-edge tile is reduced into a 256-wide PSUM
    destination window via PE matmul with a per-tile one-hot matrix S
    (S[e, slot] = 1/deg[dst_e] if slot == dst_e's window slot) built on the
    vector engine from per-edge slot/weight sidebands.
  - Dense part out = mean @ Wl + b + x @ Wr runs on PE per window with
    channel-major rhs; L2 normalization via PE transpose + ACT square/accum.
  - Between layers the 8 per-core node slices are AllGathered (fp16) into a
    replicated full table for the next layer's gathers.
  - The instruction stream is identical on all 8 cores (SPMD); all per-core
    variation lives in input data (indices, slots, weights). Tile counts per
    (window, chunk) group are padded to the max over cores.
"""

import math

import numpy as np

N_NODES = 100000
N_EDGES = 1600000
IN_C, HID_C, OUT_C = 128, 128, 64
EPS = 1e-12

N_CORES = 8
NPC = N_NODES // N_CORES        # nodes per core
WIN = 256                       # psum window width (dst slots)
N_CHUNKS = 4                    # source chunks (int16 index limit)
CHUNK_ROWS = N_NODES // N_CHUNKS
T_CALL = 8                      # tiles per dma_gather call (1024 idxs: HW SWDGE per-call cap)
P = 128

_CACHE = {}
TRACE = False          # set True (e.g. from test.py) to capture an NTFF trace
LAST_RESULT = None     # BassKernelResults of the most recent run


# --------------------------------------------------------------------------
# Host-side preprocessing: edge sort, uniform tile layout, sideband packing
# --------------------------------------------------------------------------

def _wrap_idx(flat: np.ndarray) -> np.ndarray:
    """Pack a flat int16 index list (len % 16 == 0) into the dma_gather
    wrapped layout [16, n/16] replicated to 128 partitions."""
    n = len(flat)
    arr = flat.reshape(n // 16, 16).T.astype(np.int16)
    return np.tile(arr, (8, 1))


def _preprocess(edge_index: np.ndarray):
    src = np.ascontiguousarray(edge_index[0]).astype(np.int64)
    dst = np.ascontiguousarray(edge_index[1]).astype(np.int64)
    deg = np.bincount(dst, minlength=N_NODES)
    winv = (1.0 / np.maximum(deg, 1.0)).astype(np.float32)

    core = dst // NPC
    w_in = (dst % NPC) // WIN
    chunk = src // CHUNK_ROWS
    NW = math.ceil(NPC / WIN)

    # group counts per (core, chunk, window)
    counts = np.zeros((N_CORES, N_CHUNKS, NW), dtype=np.int64)
    np.add.at(counts, (core, chunk, w_in), 1)
    # uniform tile budget per (window, chunk): max over cores
    B = np.ceil(counts.max(axis=0) / P).astype(np.int64)  # [N_CHUNKS, NW]

    # order edges by (core, chunk, window, dst) — chunk-major gather runs
    order = np.lexsort((dst, w_in, chunk, core))
    src_s, dst_s = src[order], dst[order]
    co_s, ch_s, w_s = core[order], chunk[order], w_in[order]

    NT_chunk = B.sum(axis=1)            # tiles per chunk-run  [N_CHUNKS]
    NT_total = int(NT_chunk.sum())      # tiles per core per layer
    pad_edges = NT_total * P            # padded edges per core

    # chunk-run tile offset of (c, w): tiles of chunk c laid out window-major
    cumB = np.zeros((N_CHUNKS, NW + 1), dtype=np.int64)
    cumB[:, 1:] = np.cumsum(B, axis=1)
    chunk_off = np.zeros(N_CHUNKS + 1, dtype=np.int64)
    chunk_off[1:] = np.cumsum(NT_chunk)

    idx_cols = pad_edges // 16
    idx_all = np.zeros((N_CORES, 128, idx_cols), dtype=np.int16)
    slot_all = np.zeros((N_CORES, 128, NT_total), dtype=np.float32)
    wgt_all = np.zeros((N_CORES, 128, NT_total), dtype=np.float32)

    # processing-order global tile index g for (w, c, j):
    # tiles ordered by (w, c, j)
    g_off = np.zeros((NW, N_CHUNKS), dtype=np.int64)
    g = 0
    for w in range(NW):
        for c in range(N_CHUNKS):
            g_off[w, c] = g
            g += int(B[c, w])
    assert g == NT_total

    for k in range(N_CORES):
        sel = co_s == k
        sk, dk, ck, wk = src_s[sel], dst_s[sel], ch_s[sel], w_s[sel]
        # per-(c,w) boundaries within this core's slice (sorted by c, w)
        cnt = np.zeros((N_CHUNKS, NW), dtype=np.int64)
        np.add.at(cnt, (ck, wk), 1)
        starts = np.zeros((N_CHUNKS, NW), dtype=np.int64)
        flat_sizes = cnt.reshape(-1)
        flat_starts = np.zeros_like(flat_sizes)
        flat_starts[1:] = np.cumsum(flat_sizes)[:-1]
        starts = flat_starts.reshape(N_CHUNKS, NW)

        idx_pad = np.zeros(pad_edges, dtype=np.int16)
        slot_pad = np.zeros(pad_edges, dtype=np.float32)
        wgt_pad = np.zeros(pad_edges, dtype=np.float32)
        for c in range(N_CHUNKS):
            for w in range(NW):
                n = int(cnt[c, w])
                bt = int(B[c, w])
                if bt == 0:
                    assert n == 0
                    continue
                s0 = int(starts[c, w])
                e_src = sk[s0 : s0 + n]
                e_dst = dk[s0 : s0 + n]
                p0 = (chunk_off[c] + cumB[c, w]) * P
                idx_pad[p0 : p0 + n] = (e_src - c * CHUNK_ROWS).astype(np.int16)
                slot_pad[p0 : p0 + n] = (e_dst - k * NPC - w * WIN).astype(np.float32)
                wgt_pad[p0 : p0 + n] = winv[e_dst].astype(np.float32)
                # pads: idx 0 (valid row), wgt 0, slot 0

        idx_all[k] = _wrap_idx(idx_pad)
        # tile sidebands in processing order (w, c, j)
        sp = slot_pad.reshape(NT_total, P)  # chunk-run order tiles
        wp = wgt_pad.reshape(NT_total, P)
        for w in range(NW):
            for c in range(N_CHUNKS):
                bt = int(B[c, w])
                if bt == 0:
                    continue
                pos0 = chunk_off[c] + cumB[c, w]
                gg = g_off[w, c]
                slot_all[k, :, gg : gg + bt] = sp[pos0 : pos0 + bt].T
                wgt_all[k, :, gg : gg + bt] = wp[pos0 : pos0 + bt].T

    # gather call layout per chunk: blocks of T_CALL tiles
    calls = []  # list of (chunk, tile_start_in_chunkrun, ntiles)
    for c in range(N_CHUNKS):
        t = 0
        while t < NT_chunk[c]:
            nt = int(min(T_CALL, NT_chunk[c] - t))
            calls.append((c, t, nt))
            t += nt

    struct = {
        "NW": NW,
        "B": B,
        "cumB": cumB,
        "chunk_off": chunk_off,
        "g_off": g_off,
        "NT_total": NT_total,
        "idx_cols": idx_cols,
        "calls": calls,
    }
    return struct, idx_all, slot_all, wgt_all


# --------------------------------------------------------------------------
# Device program
# --------------------------------------------------------------------------

def _build_program(struct):
    import concourse.bacc as bacc
    import concourse.bass as bass
    import concourse.tile as tile
    from concourse import mybir
    from concourse.masks import make_identity

    fp16 = mybir.dt.float16
    f32 = mybir.dt.float32

    NW = struct["NW"]
    B = struct["B"]
    cumB = struct["cumB"]
    chunk_off = struct["chunk_off"]
    g_off = struct["g_off"]
    NT_total = struct["NT_total"]
    idx_cols = struct["idx_cols"]
    calls = struct["calls"]

    nc = bacc.Bacc("TRN2", num_devices=N_CORES)

    xg = nc.dram_tensor("xg", [N_NODES, IN_C], fp16, kind="ExternalInput")
    xt = nc.dram_tensor("xt", [P, NPC], fp16, kind="ExternalInput")
    idx_t = nc.dram_tensor("idx", [128, idx_cols], mybir.dt.int16, kind="ExternalInput")
    slot_t = nc.dram_tensor("slot", [128, NT_total], f32, kind="ExternalInput")
    wgt_t = nc.dram_tensor("wgt", [128, NT_total], f32, kind="ExternalInput")
    wls, bls, wrs = [], [], []
    dims = [(IN_C, HID_C), (HID_C, HID_C), (HID_C, OUT_C)]
    for i, (din, dout) in enumerate(dims):
        wls.append(nc.dram_tensor(f"wl{i}", [din, dout], fp16, kind="ExternalInput"))
        bls.append(nc.dram_tensor(f"bl{i}", [dout, 1], f32, kind="ExternalInput"))
        wrs.append(nc.dram_tensor(f"wr{i}", [din, dout], fp16, kind="ExternalInput"))
    out_t = nc.dram_tensor("out", [NPC, OUT_C], f32, kind="ExternalOutput")

    # inter-layer buffers
    cc_in = [
        nc.dram_tensor(f"cc{i}_in", [NPC, HID_C], fp16, kind="Internal")
        for i in range(2)
    ]
    h_full = [
        nc.dram_tensor(
            f"h{i}_full", [N_NODES, HID_C], fp16, kind="Internal", addr_space="Shared"
        )
        for i in range(2)
    ]
    h_t = [
        nc.dram_tensor(f"h{i}t", [P, NPC], fp16, kind="Internal") for i in range(2)
    ]

    rg = [list(range(N_CORES))]

    with tile.TileContext(nc) as tc:
        with (
            tc.tile_pool(name="const", bufs=1) as cpool,
            tc.tile_pool(name="msg", bufs=2) as mpool,
            tc.tile_pool(name="work", bufs=3) as pool,
            tc.tile_pool(name="spool", bufs=4) as spool,
            tc.tile_pool(name="psum", bufs=2, space="PSUM") as ppool,
        ):
            # constants
            iota_i = cpool.tile([128, WIN], mybir.dt.int32)
            nc.gpsimd.iota(iota_i[:], pattern=[[1, WIN]], base=0, channel_multiplier=0)
            iota_f = cpool.tile([128, WIN], fp16)
            nc.vector.tensor_copy(iota_f[:], iota_i[:])
            ident32 = cpool.tile([128, 128], f32)
            make_identity(nc, ident32[:])
            ident16 = cpool.tile([128, 128], fp16)
            nc.vector.tensor_copy(ident16[:], ident32[:])

            idx_sb = cpool.tile([128, idx_cols], mybir.dt.int16)
            nc.sync.dma_start(idx_sb[:], idx_t[:])
            slot_sb = cpool.tile([128, NT_total], f32)
            nc.sync.dma_start(slot_sb[:], slot_t[:])
            wgt_sb = cpool.tile([128, NT_total], f32)
            nc.sync.dma_start(wgt_sb[:], wgt_t[:])

            wl_sb, bl_sb, wr_sb = [], [], []
            for i, (din, dout) in enumerate(dims):
                wl = cpool.tile([din, dout], fp16, tag=f"wl{i}")
                nc.sync.dma_start(wl[:], wls[i][:])
                bl = cpool.tile([dout, 1], f32, tag=f"bl{i}")
                nc.sync.dma_start(bl[:], bls[i][:])
                wr = cpool.tile([din, dout], fp16, tag=f"wr{i}")
                nc.sync.dma_start(wr[:], wrs[i][:])
                wl_sb.append(wl)
                bl_sb.append(bl)
                wr_sb.append(wr)

            for L in range(3):
                table = [xg, h_full[0], h_full[1]][L]
                xtab = [xt, h_t[0], h_t[1]][L]
                co = dims[L][1]

                # gather call stream state
                call_bufs = {}   # call index (in `calls`) -> sbuf tile
                next_call = 0
                covered = [0] * N_CHUNKS  # tiles covered per chunk-run
                call_base = {}
                for k2, (cc2, tt0, _nt) in enumerate(calls):
                    call_base.setdefault(cc2, k2)

                def emit_call(ci):
                    c, t0, nt = calls[ci]
                    buf = mpool.tile([128, T_CALL, 128], fp16, tag=f"g{c}")
                    col0 = (chunk_off[c] + t0) * P // 16
                    ncols = nt * P // 16
                    nc.gpsimd.dma_gather(
                        buf[:, :nt, :],
                        table[c * CHUNK_ROWS : (c + 1) * CHUNK_ROWS, :],
                        idx_sb[:, col0 : col0 + ncols],
                        nt * P,
                        nt * P,
                        128,
                    )
                    return buf

                for w in range(NW):
                    need = [int(cumB[c, w + 1]) for c in range(N_CHUNKS)]
                    while any(covered[c] < need[c] for c in range(N_CHUNKS)):
                        c, t0, nt = calls[next_call]
                        call_bufs[next_call] = emit_call(next_call)
                        covered[c] = t0 + nt
                        next_call += 1

                    wn = min(WIN, NPC - w * WIN)
                    psum = ppool.tile([128, WIN], f32, tag="agg")
                    nc.vector.memset(psum[:], 0.0)
                    ntiles_w = int(sum(B[c, w] for c in range(N_CHUNKS)))
                    done = 0
                    for c in range(N_CHUNKS):
                        bt = int(B[c, w])
                        for j in range(bt):
                            g = int(g_off[w, c]) + j
                            pos = int(cumB[c, w]) + j  # tile pos in chunk-run
                            gci = call_base[c] + pos // T_CALL
                            buf = call_bufs[gci]
                            t_in = pos % T_CALL
                            s_tile = spool.tile([128, WIN], fp16, tag="s")
                            nc.vector.tensor_scalar(
                                out=s_tile[:],
                                in0=iota_f[:],
                                scalar1=slot_sb[:, g : g + 1],
                                scalar2=wgt_sb[:, g : g + 1],
                                op0=mybir.AluOpType.is_equal,
                                op1=mybir.AluOpType.mult,
                            )
                            done += 1
                            nc.tensor.matmul(
                                psum[:],
                                lhsT=buf[:, t_in, :],
                                rhs=s_tile[:],
                                start=False,
                                stop=(done == ntiles_w),
                                skip_group_check=True,
                            )

                    # ---- dense phase for window w ----
                    meanT = pool.tile([128, WIN], fp16, tag="meanT")
                    nc.vector.tensor_copy(meanT[:, :wn], psum[:, :wn])
                    xw = pool.tile([128, WIN], fp16, tag="xw")
                    nc.sync.dma_start(xw[:, :wn], xtab[:, w * WIN : w * WIN + wn])
                    psum2 = ppool.tile([co, WIN], f32, tag="dense")
                    nc.tensor.matmul(
                        psum2[:, :wn], lhsT=wl_sb[L][:], rhs=meanT[:, :wn],
                        start=True, stop=False, skip_group_check=True,
                    )
                    nc.tensor.matmul(
                        psum2[:, :wn], lhsT=wr_sb[L][:], rhs=xw[:, :wn],
                        start=False, stop=True, skip_group_check=True,
                    )
                    dsb = pool.tile([co, WIN], f32, tag="dsb")
                    nc.vector.tensor_scalar(
                        out=dsb[:, :wn], in0=psum2[:, :wn],
                        scalar1=bl_sb[L][:], scalar2=None,
                        op0=mybir.AluOpType.add,
                    )
                    n_sub = math.ceil(wn / 128)
                    for sub in range(n_sub):
                        bs = min(128, wn - sub * 128)
                        n0 = w * WIN + sub * 128
                        psum3 = ppool.tile([128, 128], f32, tag="tp")
                        nc.tensor.transpose(
                            psum3[:bs, :co],
                            dsb[:, sub * 128 : sub * 128 + bs],
                            ident32[:co, :co],
                        )
                        sq = pool.tile([128, 128], f32, tag="sq")
                        ssq = pool.tile([128, 1], f32, tag="ssq")
                        nc.scalar.activation(
                            sq[:bs, :co], psum3[:bs, :co],
                            mybir.ActivationFunctionType.Square,
                            accum_out=ssq[:bs, :],
                        )
                        nrm = pool.tile([128, 1], f32, tag="nrm")
                        nc.scalar.activation(
                            nrm[:bs, :], ssq[:bs, :],
                            mybir.ActivationFunctionType.Sqrt,
                        )
                        nc.vector.tensor_scalar(
                            out=nrm[:bs, :], in0=nrm[:bs, :], scalar1=float(EPS),
                            scalar2=None, op0=mybir.AluOpType.max,
                        )
                        rinv = pool.tile([128, 1], f32, tag="rinv")
                        nc.vector.reciprocal(rinv[:bs, :], nrm[:bs, :])
                        if L < 2:
                            hn = pool.tile([128, 128], fp16, tag="hn")
                            nc.scalar.activation(
                                hn[:bs, :co], psum3[:bs, :co],
                                mybir.ActivationFunctionType.Relu,
                                scale=rinv[:bs, :],
                            )
                            nc.sync.dma_start(cc_in[L][n0 : n0 + bs, :], hn[:bs, :co])
                            psum4 = ppool.tile([128, 128], fp16, tag="tp2")
                            nc.tensor.transpose(
                                psum4[:co, :bs], hn[:bs, :co], ident16[:bs, :bs]
                            )
                            hts = pool.tile([128, 128], fp16, tag="hts")
                            nc.vector.tensor_copy(hts[:co, :bs], psum4[:co, :bs])
                            nc.sync.dma_start(
                                h_t[L][:, n0 : n0 + bs], hts[:co, :bs]
                            )
                        else:
                            hn = pool.tile([128, 64], f32, tag="hnf")
                            nc.vector.tensor_scalar(
                                out=hn[:bs, :co], in0=psum3[:bs, :co],
                                scalar1=rinv[:bs, :], scalar2=None,
                                op0=mybir.AluOpType.mult,
                            )
                            nc.sync.dma_start(out_t[n0 : n0 + bs, :], hn[:bs, :co])

                if L < 2:
                    nc.gpsimd.collective_compute(
                        "AllGather",
                        mybir.AluOpType.bypass,
                        replica_groups=rg,
                        ins=[cc_in[L][:]],
                        outs=[h_full[L][:]],
                    )
    nc.compile()
    return nc


# --------------------------------------------------------------------------
# Entry point
# --------------------------------------------------------------------------

def kernel(**inputs) -> np.ndarray:
    from concourse.bass_utils import run_bass_kernel_spmd

    x = np.asarray(inputs["x"], dtype=np.float32)
    edge_index = np.asarray(inputs["edge_index"])

    struct, idx_all, slot_all, wgt_all = _preprocess(edge_index)

    key = ("prog", struct["NT_total"], struct["idx_cols"], tuple(struct["chunk_off"]))
    if key not in _CACHE:
        _CACHE[key] = _build_program(struct)
    nc = _CACHE[key]

    xg = x.astype(np.float16)
    in_maps = []
    for k in range(N_CORES):
        m = {
            "xg": xg,
            "xt": np.ascontiguousarray(
                x[k * NPC : (k + 1) * NPC, :].T.astype(np.float16)
            ),
            "idx": idx_all[k],
            "slot": slot_all[k],
            "wgt": wgt_all[k],
        }
        for i in range(3):
            m[f"wl{i}"] = np.asarray(inputs[f"Wl{i}"], dtype=np.float16)
            m[f"bl{i}"] = np.asarray(inputs[f"bl{i}"], dtype=np.float32).reshape(-1, 1)
            m[f"wr{i}"] = np.asarray(inputs[f"Wr{i}"], dtype=np.float16)
        in_maps.append(m)

    res = run_bass_kernel_spmd(
        nc, in_maps, core_ids=list(range(N_CORES)), trace=TRACE
    )
    global LAST_RESULT
    LAST_RESULT = res
    out = np.concatenate([res.results[k]["out"] for k in range(N_CORES)], axis=0)
    return out.astype(np.float32)



# revision 17
# speedup vs baseline: 1.6890x; 1.6890x over previous
"""3-layer GraphSAGE (mean aggr + L2 norm) on 8 Trainium2 NeuronCores.

Strategy v4 (graph/data parallel, dst-sharded, PE-built batched one-hots):
  - Nodes are partitioned into 8 contiguous ranges (12500 per core); each core
    computes all three layers for its destination range.
  - Mean aggregation is a stream of one-hot scatter matmuls into 128-wide PSUM
    destination windows: psum_agg[chan, slot] += msgs[e, chan] * S[e, slot],
    with S[e, slot] = onehot(slot_e) (pure 0/1; degree norm applied later).
  - S tiles are built ON the PE: psum_s[e, i] = 1 - (slot_e - i)^2 via an
    exact K=5 integer polynomial matmul (sides {1,1,slot,slot^2>>7,slot^2&127}
    x poly columns; all inputs fp16-exact, accumulation fp32-exact), 4 tiles
    per PSUM bank. A single batched Relu pass (alternating ACT / DVE) yields
    the exact 0/1 one-hots in fp16 SBUF. Pad edges carry slot=-1 -> zero row.
    This avoids streaming precomputed S tiles from HBM (the DMA engines are
    the bottleneck at ~78% occupancy) at a small PE cost.
  - Degree norm (winv = 1/max(deg,1), per dst node) applies in the dense
    phase, node-major: psumA[node, co] = aggT @ Wl; ta = psumA * winv (DVE,
    winv is a per-partition scalar); psumD[node, co] = xT @ Wr + ones^T @ b
    + I @ ta (identity matmul folds ta back into the PSUM accumulation).
    The L2 norm reads psumD directly; no output transpose.
  - Messages h[src] are gathered from a replicated fp16 node table in HBM
    with dma_gather (int16 indices -> 4 source chunks of 25000 rows), 8 tiles
    (1024 rows) per call, alternating between 2 SWDGE queues with a 64KB
    descriptor carveout so generation overlaps transfer. Edges are sorted by
    src within each (chunk, window) cell for HBM locality.
  - Between layers the 8 per-core node slices are AllGathered (fp16) into a
    replicated full table for the next layer's gathers.
  - SPMD: instruction stream identical on all 8 cores; per-core variation is
    input data only. Tile counts per (window, chunk) are padded to the max
    over cores.
"""

import math

import numpy as np

N_NODES = 100000
N_EDGES = 1600000
IN_C, HID_C, OUT_C = 128, 128, 64
EPS = 1e-12

N_CORES = 8
NPC = N_NODES // N_CORES        # nodes per core
WIN = 128                       # psum window width (dst slots)
N_CHUNKS = 4                    # source chunks (int16 index limit)
CHUNK_ROWS = N_NODES // N_CHUNKS
T_CALL = 8                      # tiles per dma_gather call (1024 idxs: HW SWDGE per-call cap)
P = 128
NW = math.ceil(NPC / WIN)       # windows per core
FB = 4                          # S tiles per finalize batch (one PSUM bank)

_CACHE = {}
TRACE = False          # set True (e.g. from test.py) to capture an NTFF trace
LAST_RESULT = None     # BassKernelResults of the most recent run


# --------------------------------------------------------------------------
# Host-side preprocessing: edge sort, uniform tile layout, sideband packing
# --------------------------------------------------------------------------

def _wrap_idx(flat: np.ndarray) -> np.ndarray:
    """Pack a flat int16 index list (len % 16 == 0) into the dma_gather
    wrapped layout [16, n/16] replicated to 128 partitions."""
    n = len(flat)
    arr = flat.reshape(n // 16, 16).T.astype(np.int16)
    return np.tile(arr, (8, 1))


def _preprocess(edge_index: np.ndarray):
    src = np.ascontiguousarray(edge_index[0]).astype(np.int64)
    dst = np.ascontiguousarray(edge_index[1]).astype(np.int64)
    deg = np.bincount(dst, minlength=N_NODES)
    winv = (1.0 / np.maximum(deg, 1.0)).astype(np.float32)

    core = dst // NPC
    w_in = (dst % NPC) // WIN
    chunk = src // CHUNK_ROWS

    # group counts per (core, chunk, window)
    counts = np.zeros((N_CORES, N_CHUNKS, NW), dtype=np.int64)
    np.add.at(counts, (core, chunk, w_in), 1)
    # uniform tile budget per (chunk, window): max over cores
    B = np.ceil(counts.max(axis=0) / P).astype(np.int64)  # [N_CHUNKS, NW]

    # order edges by (core, chunk, window, src) — chunk-major gather runs;
    # src-ascending within a cell improves HBM locality of the random reads
    order = np.lexsort((src, w_in, chunk, core))
    src_s, dst_s = src[order], dst[order]
    co_s, ch_s, w_s = core[order], chunk[order], w_in[order]

    NT_chunk = B.sum(axis=1)            # tiles per chunk-run  [N_CHUNKS]
    NT_total = int(NT_chunk.sum())      # tiles per core per layer
    pad_edges = NT_total * P            # padded edges per core

    # chunk-run tile offset of (c, w)
    cumB = np.zeros((N_CHUNKS, NW + 1), dtype=np.int64)
    cumB[:, 1:] = np.cumsum(B, axis=1)
    chunk_off = np.zeros(N_CHUNKS + 1, dtype=np.int64)
    chunk_off[1:] = np.cumsum(NT_chunk)

    # processing-order tile index: tiles ordered by (w, c, j)
    ntiles_w = B.sum(axis=0)                      # [NW]
    gw_off = np.zeros(NW + 1, dtype=np.int64)
    gw_off[1:] = np.cumsum(ntiles_w)
    assert gw_off[NW] == NT_total

    # tile permutation: chunk-run order -> processing order (w, c, j)
    perm = np.zeros(NT_total, dtype=np.int64)
    g = 0
    for w in range(NW):
        for c in range(N_CHUNKS):
            bt = int(B[c, w])
            if bt == 0:
                continue
            pos0 = chunk_off[c] + cumB[c, w]
            perm[g : g + bt] = np.arange(pos0, pos0 + bt)
            g += bt
    assert g == NT_total

    idx_cols = pad_edges // 16
    idx_all = np.zeros((N_CORES, 128, idx_cols), dtype=np.int16)
    side_all = np.zeros((N_CORES, 5, NT_total * P), dtype=np.float16)
    winv_all = np.zeros((N_CORES, 128, NW), dtype=np.float32)

    for k in range(N_CORES):
        sel = co_s == k
        sk, dk, ck, wk = src_s[sel], dst_s[sel], ch_s[sel], w_s[sel]
        cnt = np.zeros((N_CHUNKS, NW), dtype=np.int64)
        np.add.at(cnt, (ck, wk), 1)
        flat_sizes = cnt.reshape(-1)
        flat_starts = np.zeros_like(flat_sizes)
        flat_starts[1:] = np.cumsum(flat_sizes)[:-1]
        starts = flat_starts.reshape(N_CHUNKS, NW)

        idx_pad = np.zeros(pad_edges, dtype=np.int16)
        slot_pad = np.full(pad_edges, -1, dtype=np.int64)   # pads: slot=-1
        for c in range(N_CHUNKS):
            for w in range(NW):
                n = int(cnt[c, w])
                if B[c, w] == 0:
                    assert n == 0
                    continue
                s0 = int(starts[c, w])
                e_src = sk[s0 : s0 + n]
                e_dst = dk[s0 : s0 + n]
                p0 = (chunk_off[c] + cumB[c, w]) * P
                idx_pad[p0 : p0 + n] = (e_src - c * CHUNK_ROWS).astype(np.int16)
                slot_pad[p0 : p0 + n] = e_dst - k * NPC - w * WIN

        idx_all[k] = _wrap_idx(idx_pad)

        # sidebands in processing order:
        # rows {1, 1, slot, slot^2>>7, slot^2 - 128*(slot^2>>7)}
        sq = slot_pad * slot_pad
        q = sq >> 7
        r = sq - 128 * q
        side_run = np.stack(
            [
                np.ones(pad_edges),
                np.ones(pad_edges),
                slot_pad.astype(np.float64),
                q.astype(np.float64),
                r.astype(np.float64),
            ]
        ).astype(np.float16)                     # [5, pad_edges] chunk-run order
        side_tiles = side_run.reshape(5, NT_total, P)
        side_all[k] = side_tiles[:, perm, :].reshape(5, NT_total * P)

        # per-node winv columns per window
        base = k * NPC + np.arange(NW)[None, :] * WIN + np.arange(128)[:, None]
        valid = base < (k + 1) * NPC
        winv_all[k][valid] = winv[np.minimum(base, N_NODES - 1)][valid]

    # gather call layout per chunk: blocks of T_CALL tiles
    calls = {c: [] for c in range(N_CHUNKS)}
    for c in range(N_CHUNKS):
        t = 0
        while t < NT_chunk[c]:
            nt = int(min(T_CALL, NT_chunk[c] - t))
            calls[c].append((t, nt))
            t += nt

    # poly columns for the S-build matmul: [5, WIN] fp16
    i = np.arange(WIN, dtype=np.int64)
    isq = i * i
    qi = isq >> 7
    ri = isq - 128 * qi
    poly = np.stack(
        [
            -128.0 * qi,
            1.0 - ri,
            2.0 * i,
            np.full(WIN, -128.0),
            np.full(WIN, -1.0),
        ]
    ).astype(np.float16)

    struct = {
        "B": B,
        "cumB": cumB,
        "chunk_off": chunk_off,
        "ntiles_w": ntiles_w,
        "gw_off": gw_off,
        "NT_total": NT_total,
        "idx_cols": idx_cols,
        "calls": calls,
    }
    return struct, idx_all, side_all, winv_all, poly


# --------------------------------------------------------------------------
# Device program
# --------------------------------------------------------------------------

def _build_program(struct):
    import concourse.bacc as bacc
    import concourse.tile as tile
    from concourse import mybir
    from concourse.masks import make_identity

    fp16 = mybir.dt.float16
    f32 = mybir.dt.float32

    B = struct["B"]
    cumB = struct["cumB"]
    chunk_off = struct["chunk_off"]
    ntiles_w = struct["ntiles_w"]
    gw_off = struct["gw_off"]
    NT_total = struct["NT_total"]
    idx_cols = struct["idx_cols"]
    calls = struct["calls"]
    ntw_max = int(ntiles_w.max())

    nc = bacc.Bacc(
        "TRN2",
        num_devices=N_CORES,
        dynamic_dma_scratch_size=65536,
        num_swdge_queues=2,
    )

    xg = nc.dram_tensor("xg", [N_NODES, IN_C], fp16, kind="ExternalInput")
    xt = nc.dram_tensor("xt", [P, NPC], fp16, kind="ExternalInput")
    idx_t = nc.dram_tensor("idx", [128, idx_cols], mybir.dt.int16, kind="ExternalInput")
    side_t = nc.dram_tensor("side", [5, NT_total * P], fp16, kind="ExternalInput")
    winv_t = nc.dram_tensor("winv", [128, NW], f32, kind="ExternalInput")
    poly_t = nc.dram_tensor("poly", [5, WIN], fp16, kind="ExternalInput")
    wls, bls, wrs = [], [], []
    dims = [(IN_C, HID_C), (HID_C, HID_C), (HID_C, OUT_C)]
    for i, (din, dout) in enumerate(dims):
        wls.append(nc.dram_tensor(f"wl{i}", [din, dout], fp16, kind="ExternalInput"))
        bls.append(nc.dram_tensor(f"bl{i}", [1, dout], fp16, kind="ExternalInput"))
        wrs.append(nc.dram_tensor(f"wr{i}", [din, dout], fp16, kind="ExternalInput"))
    out_t = nc.dram_tensor("out", [NPC, OUT_C], f32, kind="ExternalOutput")

    # inter-layer buffers
    cc_in = [
        nc.dram_tensor(f"cc{i}_in", [NPC, HID_C], fp16, kind="Internal")
        for i in range(2)
    ]
    h_full = [
        nc.dram_tensor(
            f"h{i}_full", [N_NODES, HID_C], fp16, kind="Internal", addr_space="Shared"
        )
        for i in range(2)
    ]
    h_t = [
        nc.dram_tensor(f"h{i}t", [P, NPC], fp16, kind="Internal") for i in range(2)
    ]

    rg = [list(range(N_CORES))]

    with tile.TileContext(nc) as tc:
        with (
            tc.tile_pool(name="const", bufs=1) as cpool,
            tc.tile_pool(name="msg", bufs=3) as mpool,
            tc.tile_pool(name="side", bufs=3) as sdpool,
            tc.tile_pool(name="stile", bufs=3) as spool,
            tc.tile_pool(name="work", bufs=3) as pool,
            tc.tile_pool(name="ps_s", bufs=2, space="PSUM") as pp_s,
            tc.tile_pool(name="ps_agg", bufs=2, space="PSUM") as pp_agg,
            tc.tile_pool(name="ps_ad", bufs=2, space="PSUM") as pp_ad,
            tc.tile_pool(name="ps_tp", bufs=2, space="PSUM") as pp_tp,
        ):
            # constants
            ident16 = cpool.tile([128, 128], fp16)
            make_identity(nc, ident16[:])
            ones_sb = cpool.tile([1, 128], fp16)
            nc.vector.memset(ones_sb[:], 1.0)

            idx_sb = cpool.tile([128, idx_cols], mybir.dt.int16)
            nc.sync.dma_start(idx_sb[:], idx_t[:])
            winv_sb = cpool.tile([128, NW], f32)
            nc.sync.dma_start(winv_sb[:], winv_t[:])
            poly_sb = cpool.tile([5, WIN], fp16)
            nc.sync.dma_start(poly_sb[:], poly_t[:])

            wl_sb, bl_sb, wr_sb = [], [], []
            for i, (din, dout) in enumerate(dims):
                wl = cpool.tile([din, dout], fp16, tag=f"wl{i}")
                nc.sync.dma_start(wl[:], wls[i][:])
                bl = cpool.tile([1, dout], fp16, tag=f"bl{i}")
                nc.sync.dma_start(bl[:], bls[i][:])
                wr = cpool.tile([din, dout], fp16, tag=f"wr{i}")
                nc.sync.dma_start(wr[:], wrs[i][:])
                wl_sb.append(wl)
                bl_sb.append(bl)
                wr_sb.append(wr)

            fin_parity = 0  # alternate batched Relu finalize between ACT and DVE

            for L in range(3):
                table = [xg, h_full[0], h_full[1]][L]
                xtab = [xt, h_t[0], h_t[1]][L]
                co = dims[L][1]

                # gather call stream state
                call_bufs = {}                  # (chunk, call#) -> sbuf tile
                next_call = [0] * N_CHUNKS
                covered = [0] * N_CHUNKS        # tiles covered per chunk-run

                def emit_call(c):
                    ci = next_call[c]
                    t0, nt = calls[c][ci]
                    buf = mpool.tile([128, T_CALL, 128], fp16, tag=f"g{c}")
                    col0 = (chunk_off[c] + t0) * P // 16
                    ncols = nt * P // 16
                    nc.gpsimd.dma_gather(
                        buf[:, :nt, :],
                        table[c * CHUNK_ROWS : (c + 1) * CHUNK_ROWS, :],
                        idx_sb[:, col0 : col0 + ncols],
                        nt * P,
                        nt * P,
                        128,
                        queue_num=(c + ci) % 2,
                    )
                    call_bufs[(c, ci)] = buf
                    next_call[c] = ci + 1
                    covered[c] = t0 + nt

                # sideband group loader (one DMA per window)
                side_bufs = {}

                def load_side(w):
                    ntw = int(ntiles_w[w])
                    sb = sdpool.tile([5, ntw_max * P], fp16, tag="side")
                    off = int(gw_off[w]) * P
                    nc.sync.dma_start(
                        sb[:, : ntw * P], side_t[:, off : off + ntw * P]
                    )
                    side_bufs[w] = sb

                load_side(0)

                for w in range(NW):
                    wn = min(WIN, NPC - w * WIN)
                    ntw = int(ntiles_w[w])
                    if w + 1 < NW:
                        load_side(w + 1)
                    for c in range(N_CHUNKS):
                        while covered[c] < int(cumB[c, w + 1]):
                            emit_call(c)

                    side_sb = side_bufs.pop(w)
                    psum_agg = pp_agg.tile([128, WIN], f32, tag="agg")

                    # tiles of this window in (chunk, j) order
                    tiles = []
                    for c in range(N_CHUNKS):
                        for j in range(int(B[c, w])):
                            pos = int(cumB[c, w]) + j
                            tiles.append((c, pos))

                    done = 0
                    for b0 in range(0, ntw, FB):
                        bt = tiles[b0 : b0 + FB]
                        nb = len(bt)
                        psum_s = pp_s.tile([128, FB * WIN], f32, tag="s")
                        for jj in range(nb):
                            li = b0 + jj
                            nc.tensor.matmul(
                                psum_s[:, jj * WIN : (jj + 1) * WIN],
                                lhsT=side_sb[:, li * P : (li + 1) * P],
                                rhs=poly_sb[:],
                                start=True,
                                stop=True,
                                skip_group_check=True,
                            )
                        s_sb = spool.tile([128, FB * WIN], fp16, tag="s")
                        if fin_parity == 0:
                            nc.scalar.activation(
                                s_sb[:, : nb * WIN],
                                psum_s[:, : nb * WIN],
                                mybir.ActivationFunctionType.Relu,
                            )
                        else:
                            nc.vector.tensor_scalar(
                                out=s_sb[:, : nb * WIN],
                                in0=psum_s[:, : nb * WIN],
                                scalar1=0.0,
                                scalar2=None,
                                op0=mybir.AluOpType.max,
                            )
                        fin_parity ^= 1
                        for jj in range(nb):
                            c, pos = bt[jj]
                            gci = pos // T_CALL
                            t_in = pos % T_CALL
                            buf = call_bufs[(c, gci)]
                            done += 1
                            nc.tensor.matmul(
                                psum_agg[:],
                                lhsT=buf[:, t_in, :],
                                rhs=s_sb[:, jj * WIN : (jj + 1) * WIN],
                                start=(done == 1),
                                stop=(done == ntw),
                                skip_group_check=True,
                            )

                    # ---- dense phase for window w (node-major) ----
                    # psum_ad region 0: A = aggT @ Wl ; region 1: D = output
                    aggT = pool.tile([128, WIN], fp16, tag="aggT")
                    nc.vector.tensor_copy(aggT[:, :wn], psum_agg[:, :wn])
                    xw = pool.tile([128, WIN], fp16, tag="xw")
                    nc.sync.dma_start(xw[:, :wn], xtab[:, w * WIN : w * WIN + wn])
                    psum_ad = pp_ad.tile([128, 2, 128], f32, tag="ad")
                    nc.tensor.matmul(
                        psum_ad[:wn, 0, :co], lhsT=aggT[:, :wn], rhs=wl_sb[L][:],
                        start=True, stop=True, skip_group_check=True,
                    )
                    # ta = A * winv (winv per-partition = per-node)
                    ta = pool.tile([128, 128], fp16, tag="ta")
                    nc.vector.tensor_scalar(
                        out=ta[:wn, :co], in0=psum_ad[:wn, 0, :co],
                        scalar1=winv_sb[:wn, w : w + 1], scalar2=None,
                        op0=mybir.AluOpType.mult,
                    )
                    nc.tensor.matmul(
                        psum_ad[:wn, 1, :co], lhsT=xw[:, :wn], rhs=wr_sb[L][:],
                        start=True, stop=False, skip_group_check=True,
                    )
                    nc.tensor.matmul(
                        psum_ad[:wn, 1, :co], lhsT=ones_sb[:, :wn], rhs=bl_sb[L][:],
                        start=False, stop=False, skip_group_check=True,
                    )
                    nc.tensor.matmul(
                        psum_ad[:wn, 1, :co], lhsT=ident16[:wn, :wn], rhs=ta[:wn, :co],
                        start=False, stop=True, skip_group_check=True,
                    )
                    # L2 norm directly on psum_ad region 1
                    sq = pool.tile([128, 128], f32, tag="sq")
                    ssq = pool.tile([128, 1], f32, tag="ssq")
                    nc.scalar.activation(
                        sq[:wn, :co], psum_ad[:wn, 1, :co],
                        mybir.ActivationFunctionType.Square,
                        accum_out=ssq[:wn, :],
                    )
                    nrm = pool.tile([128, 1], f32, tag="nrm")
                    nc.scalar.activation(
                        nrm[:wn, :], ssq[:wn, :],
                        mybir.ActivationFunctionType.Sqrt,
                    )
                    nc.vector.tensor_scalar(
                        out=nrm[:wn, :], in0=nrm[:wn, :], scalar1=float(EPS),
                        scalar2=None, op0=mybir.AluOpType.max,
                    )
                    rinv = pool.tile([128, 1], f32, tag="rinv")
                    nc.vector.reciprocal(rinv[:wn, :], nrm[:wn, :])
                    n0 = w * WIN
                    if L < 2:
                        hn = pool.tile([128, 128], fp16, tag="hn")
                        nc.scalar.activation(
                            hn[:wn, :co], psum_ad[:wn, 1, :co],
                            mybir.ActivationFunctionType.Relu,
                            scale=rinv[:wn, :],
                        )
                        nc.sync.dma_start(cc_in[L][n0 : n0 + wn, :], hn[:wn, :co])
                        psum_tp = pp_tp.tile([128, 128], fp16, tag="tp")
                        nc.tensor.transpose(
                            psum_tp[:co, :wn], hn[:wn, :co], ident16[:wn, :wn]
                        )
                        hts = pool.tile([128, 128], fp16, tag="hts")
                        nc.vector.tensor_copy(hts[:co, :wn], psum_tp[:co, :wn])
                        nc.sync.dma_start(h_t[L][:, n0 : n0 + wn], hts[:co, :wn])
                    else:
                        hn = pool.tile([128, 64], f32, tag="hnf")
                        nc.vector.tensor_scalar(
                            out=hn[:wn, :co], in0=psum_ad[:wn, 1, :co],
                            scalar1=rinv[:wn, :], scalar2=None,
                            op0=mybir.AluOpType.mult,
                        )
                        nc.sync.dma_start(out_t[n0 : n0 + wn, :], hn[:wn, :co])

                if L < 2:
                    nc.gpsimd.collective_compute(
                        "AllGather",
                        mybir.AluOpType.bypass,
                        replica_groups=rg,
                        ins=[cc_in[L][:]],
                        outs=[h_full[L][:]],
                    )
    nc.compile()
    return nc


# --------------------------------------------------------------------------
# Entry point
# --------------------------------------------------------------------------

def kernel(**inputs) -> np.ndarray:
    from concourse.bass_utils import run_bass_kernel_spmd

    x = np.asarray(inputs["x"], dtype=np.float32)
    edge_index = np.asarray(inputs["edge_index"])

    struct, idx_all, side_all, winv_all, poly = _preprocess(edge_index)

    key = ("prog4", struct["NT_total"], struct["idx_cols"],
           tuple(struct["chunk_off"]))
    if key not in _CACHE:
        _CACHE[key] = _build_program(struct)
    nc = _CACHE[key]

    xg = x.astype(np.float16)
    in_maps = []
    for k in range(N_CORES):
        m = {
            "xg": xg,
            "xt": np.ascontiguousarray(
                x[k * NPC : (k + 1) * NPC, :].T.astype(np.float16)
            ),
            "idx": idx_all[k],
            "side": side_all[k],
            "winv": winv_all[k],
            "poly": poly,
        }
        for i in range(3):
            m[f"wl{i}"] = np.asarray(inputs[f"Wl{i}"], dtype=np.float16)
            m[f"bl{i}"] = np.asarray(inputs[f"bl{i}"], dtype=np.float16).reshape(1, -1)
            m[f"wr{i}"] = np.asarray(inputs[f"Wr{i}"], dtype=np.float16)
        in_maps.append(m)

    res = run_bass_kernel_spmd(
        nc, in_maps, core_ids=list(range(N_CORES)), trace=TRACE
    )
    global LAST_RESULT
    LAST_RESULT = res
    out = np.concatenate([res.results[k]["out"] for k in range(N_CORES)], axis=0)
    return out.astype(np.float32)


# revision 19
# speedup vs baseline: 2.5469x; 1.5079x over previous
"""3-layer GraphSAGE (mean aggr + L2 norm) on 8 Trainium2 NeuronCores.

Strategy v4 (graph/data parallel, dst-sharded, PE-built batched one-hots):
  - Nodes are partitioned into 8 contiguous ranges (12500 per core); each core
    computes all three layers for its destination range.
  - Mean aggregation is a stream of one-hot scatter matmuls into 128-wide PSUM
    destination windows: psum_agg[chan, slot] += msgs[e, chan] * S[e, slot],
    with S[e, slot] = onehot(slot_e) (pure 0/1; degree norm applied later).
  - S tiles are built ON the PE: psum_s[e, i] = 1 - (slot_e - i)^2 via an
    exact K=5 integer polynomial matmul (sides {1,1,slot,slot^2>>7,slot^2&127}
    x poly columns; all inputs fp16-exact, accumulation fp32-exact), 4 tiles
    per PSUM bank. A single batched Relu pass (alternating ACT / DVE) yields
    the exact 0/1 one-hots in fp16 SBUF. Pad edges carry slot=-1 -> zero row.
    This avoids streaming precomputed S tiles from HBM (the DMA engines are
    the bottleneck at ~78% occupancy) at a small PE cost.
  - Degree norm (winv = 1/max(deg,1), per dst node) applies in the dense
    phase, node-major: psumA[node, co] = aggT @ Wl; ta = psumA * winv (DVE,
    winv is a per-partition scalar); psumD[node, co] = xT @ Wr + ones^T @ b
    + I @ ta (identity matmul folds ta back into the PSUM accumulation).
    The L2 norm reads psumD directly; no output transpose.
  - Messages h[src] are gathered from a replicated fp16 node table in HBM
    with dma_gather (int16 indices -> 4 source chunks of 25000 rows), 8 tiles
    (1024 rows) per call, alternating between 2 SWDGE queues with a 64KB
    descriptor carveout so generation overlaps transfer. Edges are sorted by
    src within each (chunk, window) cell for HBM locality.
  - Between layers the 8 per-core node slices are AllGathered (fp16) into a
    replicated full table for the next layer's gathers.
  - SPMD: instruction stream identical on all 8 cores; per-core variation is
    input data only. Tile counts per (window, chunk) are padded to the max
    over cores.
"""

import math

import numpy as np

N_NODES = 100000
N_EDGES = 1600000
IN_C, HID_C, OUT_C = 128, 128, 64
EPS = 1e-12

N_CORES = 8
NPC = N_NODES // N_CORES        # nodes per core
WIN = 128                       # psum window width (dst slots)
N_CHUNKS = 4                    # source chunks (int16 index limit)
CHUNK_ROWS = N_NODES // N_CHUNKS
T_CALL = 8                      # tiles per dma_gather call (1024 idxs: HW SWDGE per-call cap)
P = 128
NW = math.ceil(NPC / WIN)       # windows per core
FB = 4                          # S tiles per finalize batch (one PSUM bank)

_CACHE = {}
TRACE = False          # set True (e.g. from test.py) to capture an NTFF trace
LAST_RESULT = None     # BassKernelResults of the most recent run


# --------------------------------------------------------------------------
# Host-side preprocessing: edge sort, uniform tile layout, sideband packing
# --------------------------------------------------------------------------

def _wrap_idx(flat: np.ndarray) -> np.ndarray:
    """Pack a flat int16 index list (len % 16 == 0) into the dma_gather
    wrapped layout [16, n/16] replicated to 128 partitions."""
    n = len(flat)
    arr = flat.reshape(n // 16, 16).T.astype(np.int16)
    return np.tile(arr, (8, 1))


def _preprocess(edge_index: np.ndarray):
    src = np.ascontiguousarray(edge_index[0]).astype(np.int64)
    dst = np.ascontiguousarray(edge_index[1]).astype(np.int64)
    deg = np.bincount(dst, minlength=N_NODES)
    winv = (1.0 / np.maximum(deg, 1.0)).astype(np.float32)

    core = dst // NPC
    w_in = (dst % NPC) // WIN
    chunk = src // CHUNK_ROWS

    # group counts per (core, chunk, window)
    counts = np.zeros((N_CORES, N_CHUNKS, NW), dtype=np.int64)
    np.add.at(counts, (core, chunk, w_in), 1)
    # uniform tile budget per (chunk, window): max over cores
    B = np.ceil(counts.max(axis=0) / P).astype(np.int64)  # [N_CHUNKS, NW]

    # order edges by (core, chunk, window, src) — chunk-major gather runs;
    # src-ascending within a cell improves HBM locality of the random reads
    order = np.lexsort((src, w_in, chunk, core))
    src_s, dst_s = src[order], dst[order]
    co_s, ch_s, w_s = core[order], chunk[order], w_in[order]

    NT_chunk = B.sum(axis=1)            # tiles per chunk-run  [N_CHUNKS]
    NT_total = int(NT_chunk.sum())      # tiles per core per layer
    pad_edges = NT_total * P            # padded edges per core

    # chunk-run tile offset of (c, w)
    cumB = np.zeros((N_CHUNKS, NW + 1), dtype=np.int64)
    cumB[:, 1:] = np.cumsum(B, axis=1)
    chunk_off = np.zeros(N_CHUNKS + 1, dtype=np.int64)
    chunk_off[1:] = np.cumsum(NT_chunk)

    # processing-order tile index: tiles ordered by (w, c, j)
    ntiles_w = B.sum(axis=0)                      # [NW]
    gw_off = np.zeros(NW + 1, dtype=np.int64)
    gw_off[1:] = np.cumsum(ntiles_w)
    assert gw_off[NW] == NT_total

    # tile permutation: chunk-run order -> processing order (w, c, j)
    perm = np.zeros(NT_total, dtype=np.int64)
    g = 0
    for w in range(NW):
        for c in range(N_CHUNKS):
            bt = int(B[c, w])
            if bt == 0:
                continue
            pos0 = chunk_off[c] + cumB[c, w]
            perm[g : g + bt] = np.arange(pos0, pos0 + bt)
            g += bt
    assert g == NT_total

    idx_cols = pad_edges // 16
    idx_all = np.zeros((N_CORES, 128, idx_cols), dtype=np.int16)
    side_all = np.zeros((N_CORES, 5, NT_total * P), dtype=np.float16)
    winv_all = np.zeros((N_CORES, 128, NW), dtype=np.float32)

    for k in range(N_CORES):
        sel = co_s == k
        sk, dk, ck, wk = src_s[sel], dst_s[sel], ch_s[sel], w_s[sel]
        cnt = np.zeros((N_CHUNKS, NW), dtype=np.int64)
        np.add.at(cnt, (ck, wk), 1)
        flat_sizes = cnt.reshape(-1)
        flat_starts = np.zeros_like(flat_sizes)
        flat_starts[1:] = np.cumsum(flat_sizes)[:-1]
        starts = flat_starts.reshape(N_CHUNKS, NW)

        idx_pad = np.zeros(pad_edges, dtype=np.int16)
        slot_pad = np.full(pad_edges, -1, dtype=np.int64)   # pads: slot=-1
        for c in range(N_CHUNKS):
            for w in range(NW):
                n = int(cnt[c, w])
                if B[c, w] == 0:
                    assert n == 0
                    continue
                s0 = int(starts[c, w])
                e_src = sk[s0 : s0 + n]
                e_dst = dk[s0 : s0 + n]
                p0 = (chunk_off[c] + cumB[c, w]) * P
                idx_pad[p0 : p0 + n] = (e_src - c * CHUNK_ROWS).astype(np.int16)
                slot_pad[p0 : p0 + n] = e_dst - k * NPC - w * WIN

        idx_all[k] = _wrap_idx(idx_pad)

        # sidebands in processing order:
        # rows {1, 1, slot, slot^2>>7, slot^2 - 128*(slot^2>>7)}
        sq = slot_pad * slot_pad
        q = sq >> 7
        r = sq - 128 * q
        side_run = np.stack(
            [
                np.ones(pad_edges),
                np.ones(pad_edges),
                slot_pad.astype(np.float64),
                q.astype(np.float64),
                r.astype(np.float64),
            ]
        ).astype(np.float16)                     # [5, pad_edges] chunk-run order
        side_tiles = side_run.reshape(5, NT_total, P)
        side_all[k] = side_tiles[:, perm, :].reshape(5, NT_total * P)

        # per-node winv columns per window
        base = k * NPC + np.arange(NW)[None, :] * WIN + np.arange(128)[:, None]
        valid = base < (k + 1) * NPC
        winv_all[k][valid] = winv[np.minimum(base, N_NODES - 1)][valid]

    # gather call layout per chunk: blocks of T_CALL tiles
    calls = {c: [] for c in range(N_CHUNKS)}
    for c in range(N_CHUNKS):
        t = 0
        while t < NT_chunk[c]:
            nt = int(min(T_CALL, NT_chunk[c] - t))
            calls[c].append((t, nt))
            t += nt

    # poly columns for the S-build matmul: [5, WIN] fp16
    i = np.arange(WIN, dtype=np.int64)
    isq = i * i
    qi = isq >> 7
    ri = isq - 128 * qi
    poly = np.stack(
        [
            -128.0 * qi,
            1.0 - ri,
            2.0 * i,
            np.full(WIN, -128.0),
            np.full(WIN, -1.0),
        ]
    ).astype(np.float16)

    struct = {
        "B": B,
        "cumB": cumB,
        "chunk_off": chunk_off,
        "ntiles_w": ntiles_w,
        "gw_off": gw_off,
        "NT_total": NT_total,
        "idx_cols": idx_cols,
        "calls": calls,
    }
    return struct, idx_all, side_all, winv_all, poly


# --------------------------------------------------------------------------
# Device program
# --------------------------------------------------------------------------

def _build_program(struct):
    import concourse.bacc as bacc
    import concourse.tile as tile
    from concourse import mybir
    from concourse.masks import make_identity

    fp16 = mybir.dt.float16
    f32 = mybir.dt.float32

    B = struct["B"]
    cumB = struct["cumB"]
    chunk_off = struct["chunk_off"]
    ntiles_w = struct["ntiles_w"]
    gw_off = struct["gw_off"]
    NT_total = struct["NT_total"]
    idx_cols = struct["idx_cols"]
    calls = struct["calls"]
    ntw_max = int(ntiles_w.max())

    nc = bacc.Bacc(
        "TRN2",
        num_devices=N_CORES,
        dynamic_dma_scratch_size=65536,
        num_swdge_queues=4,
    )

    xg = nc.dram_tensor("xg", [N_NODES, IN_C], fp16, kind="ExternalInput")
    xt = nc.dram_tensor("xt", [P, NPC], fp16, kind="ExternalInput")
    idx_t = nc.dram_tensor("idx", [128, idx_cols], mybir.dt.int16, kind="ExternalInput")
    side_t = nc.dram_tensor("side", [5, NT_total * P], fp16, kind="ExternalInput")
    winv_t = nc.dram_tensor("winv", [128, NW], f32, kind="ExternalInput")
    poly_t = nc.dram_tensor("poly", [5, WIN], fp16, kind="ExternalInput")
    wls, bls, wrs = [], [], []
    dims = [(IN_C, HID_C), (HID_C, HID_C), (HID_C, OUT_C)]
    for i, (din, dout) in enumerate(dims):
        wls.append(nc.dram_tensor(f"wl{i}", [din, dout], fp16, kind="ExternalInput"))
        bls.append(nc.dram_tensor(f"bl{i}", [1, dout], fp16, kind="ExternalInput"))
        wrs.append(nc.dram_tensor(f"wr{i}", [din, dout], fp16, kind="ExternalInput"))
    out_t = nc.dram_tensor("out", [NPC, OUT_C], f32, kind="ExternalOutput")

    # inter-layer buffers
    cc_in = [
        nc.dram_tensor(f"cc{i}_in", [NPC, HID_C], fp16, kind="Internal")
        for i in range(2)
    ]
    h_full = [
        nc.dram_tensor(
            f"h{i}_full", [N_NODES, HID_C], fp16, kind="Internal", addr_space="Shared"
        )
        for i in range(2)
    ]
    h_t = [
        nc.dram_tensor(f"h{i}t", [P, NPC], fp16, kind="Internal") for i in range(2)
    ]

    rg = [list(range(N_CORES))]

    with tile.TileContext(nc) as tc:
        with (
            tc.tile_pool(name="const", bufs=1) as cpool,
            tc.tile_pool(name="msg", bufs=3) as mpool,
            tc.tile_pool(name="side", bufs=3) as sdpool,
            tc.tile_pool(name="stile", bufs=3) as spool,
            tc.tile_pool(name="work", bufs=3) as pool,
            tc.tile_pool(name="ps_s", bufs=2, space="PSUM") as pp_s,
            tc.tile_pool(name="ps_agg", bufs=2, space="PSUM") as pp_agg,
            tc.tile_pool(name="ps_ad", bufs=2, space="PSUM") as pp_ad,
            tc.tile_pool(name="ps_tp", bufs=2, space="PSUM") as pp_tp,
        ):
            # constants
            ident16 = cpool.tile([128, 128], fp16)
            make_identity(nc, ident16[:])
            ones_sb = cpool.tile([1, 128], fp16)
            nc.vector.memset(ones_sb[:], 1.0)

            idx_sb = cpool.tile([128, idx_cols], mybir.dt.int16)
            nc.sync.dma_start(idx_sb[:], idx_t[:])
            winv_sb = cpool.tile([128, NW], f32)
            nc.sync.dma_start(winv_sb[:], winv_t[:])
            poly_sb = cpool.tile([5, WIN], fp16)
            nc.sync.dma_start(poly_sb[:], poly_t[:])

            wl_sb, bl_sb, wr_sb = [], [], []
            for i, (din, dout) in enumerate(dims):
                wl = cpool.tile([din, dout], fp16, tag=f"wl{i}")
                nc.sync.dma_start(wl[:], wls[i][:])
                bl = cpool.tile([1, dout], fp16, tag=f"bl{i}")
                nc.sync.dma_start(bl[:], bls[i][:])
                wr = cpool.tile([din, dout], fp16, tag=f"wr{i}")
                nc.sync.dma_start(wr[:], wrs[i][:])
                wl_sb.append(wl)
                bl_sb.append(bl)
                wr_sb.append(wr)

            fin_parity = 0  # alternate batched Relu finalize between ACT and DVE

            for L in range(3):
                table = [xg, h_full[0], h_full[1]][L]
                xtab = [xt, h_t[0], h_t[1]][L]
                co = dims[L][1]

                # gather call stream state
                call_bufs = {}                  # (chunk, call#) -> sbuf tile
                next_call = [0] * N_CHUNKS
                covered = [0] * N_CHUNKS        # tiles covered per chunk-run

                def emit_call(c):
                    ci = next_call[c]
                    t0, nt = calls[c][ci]
                    buf = mpool.tile([128, T_CALL, 128], fp16, tag=f"g{c}")
                    col0 = (chunk_off[c] + t0) * P // 16
                    ncols = nt * P // 16
                    nc.gpsimd.dma_gather(
                        buf[:, :nt, :],
                        table[c * CHUNK_ROWS : (c + 1) * CHUNK_ROWS, :],
                        idx_sb[:, col0 : col0 + ncols],
                        nt * P,
                        nt * P,
                        128,
                        queue_num=(c + ci) % 4,
                    )
                    call_bufs[(c, ci)] = buf
                    next_call[c] = ci + 1
                    covered[c] = t0 + nt

                # sideband group loader (one DMA per window)
                side_bufs = {}

                def load_side(w):
                    ntw = int(ntiles_w[w])
                    sb = sdpool.tile([5, ntw_max * P], fp16, tag="side")
                    off = int(gw_off[w]) * P
                    nc.sync.dma_start(
                        sb[:, : ntw * P], side_t[:, off : off + ntw * P]
                    )
                    side_bufs[w] = sb

                load_side(0)

                for w in range(NW):
                    wn = min(WIN, NPC - w * WIN)
                    ntw = int(ntiles_w[w])
                    if w + 1 < NW:
                        load_side(w + 1)
                    for c in range(N_CHUNKS):
                        while covered[c] < int(cumB[c, w + 1]):
                            emit_call(c)

                    side_sb = side_bufs.pop(w)
                    psum_agg = pp_agg.tile([128, WIN], f32, tag="agg")

                    # tiles of this window in (chunk, j) order
                    tiles = []
                    for c in range(N_CHUNKS):
                        for j in range(int(B[c, w])):
                            pos = int(cumB[c, w]) + j
                            tiles.append((c, pos))

                    done = 0
                    for b0 in range(0, ntw, FB):
                        bt = tiles[b0 : b0 + FB]
                        nb = len(bt)
                        psum_s = pp_s.tile([128, FB * WIN], f32, tag="s")
                        for jj in range(nb):
                            li = b0 + jj
                            nc.tensor.matmul(
                                psum_s[:, jj * WIN : (jj + 1) * WIN],
                                lhsT=side_sb[:, li * P : (li + 1) * P],
                                rhs=poly_sb[:],
                                start=True,
                                stop=True,
                                skip_group_check=True,
                            )
                        s_sb = spool.tile([128, FB * WIN], fp16, tag="s")
                        if fin_parity == 0:
                            nc.scalar.activation(
                                s_sb[:, : nb * WIN],
                                psum_s[:, : nb * WIN],
                                mybir.ActivationFunctionType.Relu,
                            )
                        else:
                            nc.vector.tensor_scalar(
                                out=s_sb[:, : nb * WIN],
                                in0=psum_s[:, : nb * WIN],
                                scalar1=0.0,
                                scalar2=None,
                                op0=mybir.AluOpType.max,
                            )
                        fin_parity ^= 1
                        for jj in range(nb):
                            c, pos = bt[jj]
                            gci = pos // T_CALL
                            t_in = pos % T_CALL
                            buf = call_bufs[(c, gci)]
                            done += 1
                            nc.tensor.matmul(
                                psum_agg[:],
                                lhsT=buf[:, t_in, :],
                                rhs=s_sb[:, jj * WIN : (jj + 1) * WIN],
                                start=(done == 1),
                                stop=(done == ntw),
                                skip_group_check=True,
                            )

                    # ---- dense phase for window w (node-major) ----
                    # psum_ad region 0: A = aggT @ Wl ; region 1: D = output
                    aggT = pool.tile([128, WIN], fp16, tag="aggT")
                    nc.vector.tensor_copy(aggT[:, :wn], psum_agg[:, :wn])
                    xw = pool.tile([128, WIN], fp16, tag="xw")
                    nc.sync.dma_start(xw[:, :wn], xtab[:, w * WIN : w * WIN + wn])
                    psum_ad = pp_ad.tile([128, 2, 128], f32, tag="ad")
                    nc.tensor.matmul(
                        psum_ad[:wn, 0, :co], lhsT=aggT[:, :wn], rhs=wl_sb[L][:],
                        start=True, stop=True, skip_group_check=True,
                    )
                    # ta = A * winv (winv per-partition = per-node)
                    ta = pool.tile([128, 128], fp16, tag="ta")
                    nc.vector.tensor_scalar(
                        out=ta[:wn, :co], in0=psum_ad[:wn, 0, :co],
                        scalar1=winv_sb[:wn, w : w + 1], scalar2=None,
                        op0=mybir.AluOpType.mult,
                    )
                    nc.tensor.matmul(
                        psum_ad[:wn, 1, :co], lhsT=xw[:, :wn], rhs=wr_sb[L][:],
                        start=True, stop=False, skip_group_check=True,
                    )
                    nc.tensor.matmul(
                        psum_ad[:wn, 1, :co], lhsT=ones_sb[:, :wn], rhs=bl_sb[L][:],
                        start=False, stop=False, skip_group_check=True,
                    )
                    nc.tensor.matmul(
                        psum_ad[:wn, 1, :co], lhsT=ident16[:wn, :wn], rhs=ta[:wn, :co],
                        start=False, stop=True, skip_group_check=True,
                    )
                    # L2 norm directly on psum_ad region 1
                    sq = pool.tile([128, 128], f32, tag="sq")
                    ssq = pool.tile([128, 1], f32, tag="ssq")
                    nc.scalar.activation(
                        sq[:wn, :co], psum_ad[:wn, 1, :co],
                        mybir.ActivationFunctionType.Square,
                        accum_out=ssq[:wn, :],
                    )
                    nrm = pool.tile([128, 1], f32, tag="nrm")
                    nc.scalar.activation(
                        nrm[:wn, :], ssq[:wn, :],
                        mybir.ActivationFunctionType.Sqrt,
                    )
                    nc.vector.tensor_scalar(
                        out=nrm[:wn, :], in0=nrm[:wn, :], scalar1=float(EPS),
                        scalar2=None, op0=mybir.AluOpType.max,
                    )
                    rinv = pool.tile([128, 1], f32, tag="rinv")
                    nc.vector.reciprocal(rinv[:wn, :], nrm[:wn, :])
                    n0 = w * WIN
                    if L < 2:
                        hn = pool.tile([128, 128], fp16, tag="hn")
                        nc.scalar.activation(
                            hn[:wn, :co], psum_ad[:wn, 1, :co],
                            mybir.ActivationFunctionType.Relu,
                            scale=rinv[:wn, :],
                        )
                        nc.sync.dma_start(cc_in[L][n0 : n0 + wn, :], hn[:wn, :co])
                        psum_tp = pp_tp.tile([128, 128], fp16, tag="tp")
                        nc.tensor.transpose(
                            psum_tp[:co, :wn], hn[:wn, :co], ident16[:wn, :wn]
                        )
                        hts = pool.tile([128, 128], fp16, tag="hts")
                        nc.vector.tensor_copy(hts[:co, :wn], psum_tp[:co, :wn])
                        nc.sync.dma_start(h_t[L][:, n0 : n0 + wn], hts[:co, :wn])
                    else:
                        hn = pool.tile([128, 64], f32, tag="hnf")
                        nc.vector.tensor_scalar(
                            out=hn[:wn, :co], in0=psum_ad[:wn, 1, :co],
                            scalar1=rinv[:wn, :], scalar2=None,
                            op0=mybir.AluOpType.mult,
                        )
                        nc.sync.dma_start(out_t[n0 : n0 + wn, :], hn[:wn, :co])

                if L < 2:
                    nc.gpsimd.collective_compute(
                        "AllGather",
                        mybir.AluOpType.bypass,
                        replica_groups=rg,
                        ins=[cc_in[L][:]],
                        outs=[h_full[L][:]],
                    )
    nc.compile()
    return nc


# --------------------------------------------------------------------------
# Entry point
# --------------------------------------------------------------------------

def kernel(**inputs) -> np.ndarray:
    from concourse.bass_utils import run_bass_kernel_spmd

    x = np.asarray(inputs["x"], dtype=np.float32)
    edge_index = np.asarray(inputs["edge_index"])

    struct, idx_all, side_all, winv_all, poly = _preprocess(edge_index)

    key = ("prog4", struct["NT_total"], struct["idx_cols"],
           tuple(struct["chunk_off"]))
    if key not in _CACHE:
        _CACHE[key] = _build_program(struct)
    nc = _CACHE[key]

    xg = x.astype(np.float16)
    in_maps = []
    for k in range(N_CORES):
        m = {
            "xg": xg,
            "xt": np.ascontiguousarray(
                x[k * NPC : (k + 1) * NPC, :].T.astype(np.float16)
            ),
            "idx": idx_all[k],
            "side": side_all[k],
            "winv": winv_all[k],
            "poly": poly,
        }
        for i in range(3):
            m[f"wl{i}"] = np.asarray(inputs[f"Wl{i}"], dtype=np.float16)
            m[f"bl{i}"] = np.asarray(inputs[f"bl{i}"], dtype=np.float16).reshape(1, -1)
            m[f"wr{i}"] = np.asarray(inputs[f"Wr{i}"], dtype=np.float16)
        in_maps.append(m)

    res = run_bass_kernel_spmd(
        nc, in_maps, core_ids=list(range(N_CORES)), trace=TRACE
    )
    global LAST_RESULT
    LAST_RESULT = res
    out = np.concatenate([res.results[k]["out"] for k in range(N_CORES)], axis=0)
    return out.astype(np.float32)
